# revision 1
# baseline (speedup 1.0000x reference)
"""EGNN (2-layer, graph pooling) Trainium2 SPMD kernel over 8 NeuronCores.

Edges are dst-sorted and sharded by destination-node range; per 128-node tile
the kernel gathers source-node features via indirect DMA from a projected
table, applies the edge MLP with one-hot matmul gather/scatter on the
TensorEngine, and fuses the node MLP. The layer-1 feature table is exchanged
with an AllGather; graph pooling uses host-built one-hot matmuls and a final
AllReduce. Walrus in this environment accepts one sync-wait per instruction,
so a JSON-level pass splits multi-wait instructions onto NoOp carriers.
"""
import sys
sys.path.insert(0, '/opt/trn_rl_repo')
import concourse.tile as tile_mod
from concourse.vector_clock import ScopedClock


def _patched_drain_and_barrier(self, tick_clock, wait_clock):
    nc = self.nc
    probe = nc.sync.nop(nofuse=True)
    wait_clock.add_sem_waits(probe.ins, ScopedClock({None: tick_clock.global_clock}))
    waits = list(probe.ins.sync_info.on_wait)
    probe.ins.sync_info.on_wait = []
    import concourse.mybir as mybir
    for w in waits:
        carrier = nc.sync.nop(nofuse=True)
        if carrier.ins.sync_info is None:
            carrier.ins.sync_info = mybir.SyncInfo(on_wait=[], on_update=[])
        carrier.ins.sync_info.on_wait = [w]
    nc.sync.drain()

    nc.all_engine_barrier()
    assert self.sems is not None
    popped = nc._tile_sem_poison_stack.pop()
    assert popped is self._sem_poison
    nc.clear_and_free_semaphores(list(self.sems.allocated().values()))
    nc.all_engine_barrier()


def apply_patch():
    tile_mod.TileContext._drain_and_barrier = _patched_drain_and_barrier


def _legalize_waits_json(mod: dict) -> dict:
    """Walrus in this env accepts at most ONE sync wait per instruction.
    Split extra waits onto same-engine NoOp carriers inserted just before."""
    import copy
    n_new = [0]
    for fn in mod.get('functions', []):
        for blk in fn.get('blocks', []):
            insts = blk.get('instructions', [])
            out = []
            for inst in insts:
                si = inst.get('sync_info') or {}
                waits = si.get('on_wait') or []
                if len(waits) > 1:
                    eng = inst.get('engine')
                    for w in waits[:-1]:
                        n_new[0] += 1
                        out.append({
                            'debug': inst.get('debug', 0),
                            'engine': eng, 'ins': [], 'outs': [],
                            'name': 'I-waitfix-%d' % n_new[0],
                            'opcode': 'NoOp',
                            'sync_info': {'on_update': [], 'on_wait': [w]},
                        })
                    si['on_wait'] = [waits[-1]]
                out.append(inst)
            blk['instructions'] = out
    return mod


def apply_json_patch():
    import orjson
    import concourse.bass as bass_mod
    orig = bass_mod.Bass.to_json_bytes
    def to_json_bytes(self):
        raw = orig(self)
        mod = orjson.loads(raw)
        mod = _legalize_waits_json(mod)
        return orjson.dumps(mod)
    bass_mod.Bass.to_json_bytes = to_json_bytes


import math
import numpy as np
import concourse.bass as bass
import concourse.mybir as mybir
from concourse.tile import TileContext
from concourse import bass_utils
from concourse.masks import make_identity
from concourse.tile_rust import add_dep_helper
apply_patch(); apply_json_patch()

f32 = mybir.dt.float32
i32 = mybir.dt.int32
AF = mybir.ActivationFunctionType
ALU = mybir.AluOpType
AX = mybir.AxisListType
P = 128
NC = 8
DBG_MMD = False


def host_prep(inputs, n_tiles_per_core):
    SH = n_tiles_per_core * P
    NPAD = SH * NC
    T = n_tiles_per_core
    src = np.asarray(inputs['edge_index'][0], np.int64)
    dst = np.asarray(inputs['edge_index'][1], np.int64)
    ea = np.asarray(inputs['edge_attr'], np.float32)
    order = np.argsort(dst, kind='stable')
    src, dst, ea = src[order], dst[order], ea[order]
    core_of = dst // SH
    tile_of = (dst % SH) // P

    counts = np.zeros((NC, T), np.int64)
    for c in range(NC):
        m = core_of == c
        tl, cn = np.unique(tile_of[m], return_counts=True)
        counts[c, tl] = cn
    K = np.maximum(1, np.ceil(counts / P).astype(np.int64).max(axis=0))
    offs = np.concatenate([[0], np.cumsum(K)]).astype(np.int64)
    TOT = int(offs[-1])

    src_pm = np.zeros((NC, P, TOT), np.int32)
    ea_t = np.zeros((NC, 16, TOT * P), np.float32)
    s_full = np.zeros((NC, P, TOT * P), np.float32)
    st_full = np.zeros((NC, P, TOT * P), np.float32)
    for c in range(NC):
        m = core_of == c
        s_c, d_c, e_c, t_c = src[m], dst[m], ea[m], tile_of[m]
        for t in range(T):
            mt = t_c == t
            sc, dc, ec = s_c[mt], d_c[mt], e_c[mt]
            nloc = ((dc - c * SH) - t * P).astype(np.int64)
            n_e = len(sc)
            for k in range(int(K[t])):
                blk = int(offs[t]) + k
                lo, hi = k * P, min((k + 1) * P, n_e)
                cnt = max(0, hi - lo)
                if cnt > 0:
                    src_pm[c, :cnt, blk] = sc[lo:hi]
                    ea_t[c, :, blk * P: blk * P + cnt] = ec[lo:hi].T
                    oh = np.zeros((P, P), np.float32)
                    oh[np.arange(cnt), nloc[lo:hi]] = 1.0
                    s_full[c, :, blk * P:(blk + 1) * P] = oh
                    st_full[c, :, blk * P:(blk + 1) * P] = oh.T
    # pooling one-hots + counts
    batch = np.asarray(inputs['batch'], np.int64)
    G_ = 64
    bpool = np.zeros((NC, P, T * 64), np.float32)
    N = len(batch)
    for c in range(NC):
        for t in range(T):
            for p in range(P):
                n = c * SH + t * P + p
                if n < N:
                    bpool[c, p, t * 64 + int(batch[n])] = 1.0
    cnts = np.bincount(batch, minlength=G_).astype(np.float32)
    invcnt = (1.0 / np.maximum(cnts, 1.0)).reshape(G_, 1)
    return dict(SH=SH, NPAD=NPAD, T=T, K=[int(k) for k in K],
                offs=[int(o) for o in offs], TOT=TOT, src_pm=src_pm,
                ea_t=ea_t, s_full=s_full, st_full=st_full, bpool=bpool,
                invcnt=invcnt)


def host_weights(inputs):
    w = {}
    for L in range(2):
        mw = np.asarray(inputs[f'l{L}_mlp_w'], np.float32)
        w[f'wproj{L}'] = np.concatenate([mw[0:128], mw[128:256]], axis=1)
        w[f'wea{L}'] = np.ascontiguousarray(mw[256:272])
        w[f'wrrep{L}'] = np.tile(mw[272:273], (P, 1))
        ew = np.asarray(inputs[f'l{L}_edge_w'], np.float32)
        eb = np.asarray(inputs[f'l{L}_edge_b'], np.float32)
        cw = np.asarray(inputs[f'l{L}_coord_w'], np.float32)
        cb = np.asarray(inputs[f'l{L}_coord_b'], np.float32)
        we1s = np.zeros((65, 64), np.float32)
        we1s[0:64, 0:64] = ew; we1s[64, 0:64] = eb
        w[f'we1s{L}'] = we1s
        w[f'cwrep{L}'] = np.tile(cw[:, 0][None, :], (P, 1))
        w[f'cbrep{L}'] = np.full((P, 1), cb[0], np.float32)
        n1 = np.asarray(inputs[f'l{L}_node_w1'], np.float32)
        w[f'wn1x{L}'] = np.ascontiguousarray(n1[0:128])
        w[f'wn1a{L}'] = np.ascontiguousarray(n1[128:192])
        w[f'nb1_{L}'] = np.asarray(inputs[f'l{L}_node_b1'], np.float32)[None, :]
        w[f'wn2b{L}'] = np.concatenate(
            [np.asarray(inputs[f'l{L}_node_w2'], np.float32),
             np.asarray(inputs[f'l{L}_node_b2'], np.float32)[None, :]], 0)
    w['wo1'] = np.asarray(inputs['out_w1'], np.float32)
    w['wo1b'] = np.asarray(inputs['out_b1'], np.float32)[None, :]
    w['wo2'] = np.asarray(inputs['out_w2'], np.float32)
    w['wo2b'] = np.asarray(inputs['out_b2'], np.float32)[None, :]
    return w


WSHAPES = dict(wproj0=(P, P), wproj1=(P, P), wea0=(16, 64), wea1=(16, 64),
               wrrep0=(P, 64), wrrep1=(P, 64), we1s0=(65, 64), we1s1=(65, 64),
               wn1x0=(P, 64), wn1x1=(P, 64), wn1a0=(64, 64), wn1a1=(64, 64),
               nb1_0=(1, 64), nb1_1=(1, 64), wn2b0=(65, P), wn2b1=(65, P),
               wo1=(P, P), wo1b=(1, P), wo2=(P, 32), wo2b=(1, 32),
               cwrep0=(P, 64), cwrep1=(P, 64), cbrep0=(P, 1), cbrep1=(P, 1))


def build(st):
    SH, NPAD, T, K, offs, TOT = (st['SH'], st['NPAD'], st['T'], st['K'],
                                 st['offs'], st['TOT'])
    NT_FULL = NPAD // P
    G_, OUT = 64, 32

    nc = bass.Bass("TRN2")
    dram = {}
    def din(name, shape, dt=f32):
        dram[name] = nc.dram_tensor(name, shape, dt, kind="ExternalInput")
        return dram[name]

    x_full = din('x_full', (NPAD, P))
    pos_full = din('pos_full', (NPAD, 4))
    x_own = din('x_own', (SH, P))
    pos_own_in = din('pos_own_in', (SH, 4))
    src_pm = din('src_pm', (P, TOT), i32)
    ea_td = din('ea_t', (16, TOT * P))
    s_fd = din('s_full', (P, TOT * P))
    st_fd = din('st_full', (P, TOT * P))
    bpool_d = din('bpool', (P, T * 64))
    invcnt_d = din('invcnt', (G_, 1))
    for n, shp in WSHAPES.items():
        din(n, shp)
    out_ext = nc.dram_tensor('out', (G_, OUT), f32, kind="ExternalOutput")
    dbg_h = nc.dram_tensor('dbg_h', (P, 20 * 64), f32, kind="ExternalOutput")
    dbg_sc = nc.dram_tensor('dbg_sc', (P, 20 * 68), f32, kind="ExternalOutput")
    dbg_ea = nc.dram_tensor('dbg_ea', (P, 68), f32, kind="ExternalOutput")
    dbg_rad = nc.dram_tensor('dbg_rad', (P, 20), f32, kind="ExternalOutput")
    dbg_g = nc.dram_tensor('dbg_g', (P, 20 * 68), f32, kind="ExternalOutput")

    ts0 = nc.dram_tensor('ts0', (NPAD, 68), f32, kind='ExternalOutput')
    ts1sh = nc.dram_tensor('ts1sh', (SH, 68), f32)
    ts1 = nc.dram_tensor('ts1', (NPAD, 68), f32, addr_space="Shared")
    xdp = [nc.dram_tensor('xdp0', (SH, 72), f32, kind='ExternalOutput'),
           nc.dram_tensor('xdp1', (SH, 72), f32)]
    gs_in = nc.dram_tensor('gs_in', (G_, P), f32)
    gs_out = nc.dram_tensor('gs_out', (G_, P), f32, addr_space="Shared")

    with TileContext(nc) as tc:
        with (tc.tile_pool(name="pers", bufs=1) as pers,
              tc.tile_pool(name="sb", bufs=2) as sb,
              tc.tile_pool(name="sb2", bufs=2) as sb2,
              tc.tile_pool(name="ph", bufs=4, space="PSUM") as ps_ph,
              tc.tile_pool(name="ppx", bufs=1, space="PSUM") as ps_ppx,
              tc.tile_pool(name="pagg", bufs=1, space="PSUM") as ps_pagg,
              tc.tile_pool(name="phT", bufs=1, space="PSUM") as ps_phT,
              tc.tile_pool(name="ppool", bufs=1, space="PSUM") as ps_ppool):

            ident = pers.tile([P, P], f32, name="ident", tag="ident")
            make_identity(nc, ident[:])
            ones_row = pers.tile([1, P], f32, name="ones_row", tag="ones_row")
            nc.vector.memset(ones_row[:], 1.0)
            xT_a = pers.tile([P, SH], f32, name="xT_a", tag="xT_a")
            xT_b = pers.tile([P, SH], f32, name="xT_b", tag="xT_b")
            pos_own = pers.tile([P, T * 4], f32, name="pos_own", tag="pos_own")
            posn_all = pers.tile([P, T * 4], f32, name="posn_all", tag="posn_all")
            invcnt_t = pers.tile([G_, 1], f32, name="invc", tag="invc")
            nc.sync.dma_start(invcnt_t[:], invcnt_d[:, :])
            W = {}
            for n, shp in WSHAPES.items():
                W[n] = pers.tile(list(shp), f32, name="w_" + n, tag="w_" + n)
                nc.sync.dma_start(W[n][:], dram[n][:, :])

            stageA_writes = []
            # ---------------- stage A: full-N Ts0 table ----------------
            for ti in range(NT_FULL):
                xt = sb.tile([P, P], f32, name="ax", tag="ax")
                nc.sync.dma_start(xt[:], x_full[ti * P:(ti + 1) * P, :])
                pt = sb.tile([P, 4], f32, name="ap", tag="ap")
                nc.sync.dma_start(pt[:], pos_full[ti * P:(ti + 1) * P, :])
                pxt = ps_ph.tile([P, P], f32, name="ph", tag="ph", space="PSUM")
                nc.tensor.transpose(out=pxt[:], in_=xt[:], identity=ident[:])
                xts = sb.tile([P, P], f32, name="axt", tag="axt")
                nc.scalar.activation(out=xts[:], in_=pxt[:], func=AF.Copy)
                pp = ps_ph.tile([P, 64], f32, name="ph", tag="ph", space="PSUM")
                nc.tensor.matmul(out=pp[:], lhsT=xts[:], rhs=W['wproj0'][:, 64:128],
                                 start=True, stop=True)
                tst = sb.tile([P, 68], f32, name="tst", tag="tst")
                nc.vector.tensor_copy(out=tst[:, 0:64], in_=pp[:])
                nc.vector.tensor_scalar_mul(tst[:, 64:67], pt[:, 0:3], -1.0)
                nc.vector.memset(tst[:, 67:68], 0.0)
                stageA_writes.append(nc.sync.dma_start(ts0[ti * P:(ti + 1) * P, :], tst[:]))

            # ---------------- stage A-own: resident xT + xdp0 ----------
            for t in range(T):
                xt = sb.tile([P, P], f32, name="ax", tag="ax")
                nc.sync.dma_start(xt[:], x_own[t * P:(t + 1) * P, :])
                pt = sb.tile([P, 4], f32, name="ap", tag="ap")
                nc.sync.dma_start(pt[:], pos_own_in[t * P:(t + 1) * P, :])
                nc.vector.tensor_copy(out=pos_own[:, t * 4:(t + 1) * 4], in_=pt[:])
                pxt = ps_ph.tile([P, P], f32, name="ph", tag="ph", space="PSUM")
                nc.tensor.transpose(out=pxt[:], in_=xt[:], identity=ident[:])
                nc.scalar.activation(out=xT_a[:, t * P:(t + 1) * P], in_=pxt[:], func=AF.Copy)
                pp = ps_ph.tile([P, 64], f32, name="ph", tag="ph", space="PSUM")
                nc.tensor.matmul(out=pp[:], lhsT=xT_a[:, t * P:(t + 1) * P],
                                 rhs=W['wproj0'][:, 0:64], start=True, stop=True)
                xds = sb.tile([P, 72], f32, name="xds", tag="xds")
                nc.vector.tensor_copy(out=xds[:, 0:64], in_=pp[:])
                nc.vector.tensor_copy(out=xds[:, 64:68], in_=pt[:])
                nc.vector.memset(xds[:, 68:72], 0.0)
                stageA_writes.append(nc.sync.dma_start(xdp[0][t * P:(t + 1) * P, :], xds[:]))

            # ---------------- edge + node stage, per layer --------------
            def layer(L, ts_dram, xin_T, xout_T):
                writes = []
                sfx = str(L)
                for t in range(T):
                    Kt = K[t]
                    off = offs[t]
                    ng = (Kt + 6) // 7
                    idx_t = sb2.tile([P, Kt], i32, name="idx", tag="idx")
                    nc.sync.dma_start(idx_t[:], src_pm[:, off:off + Kt])
                    g_t = sb2.tile([P, Kt * 68], f32, name="G", tag="G")
                    for k in range(Kt):
                        nc.gpsimd.indirect_dma_start(
                            out=g_t[:, k * 68:(k + 1) * 68], out_offset=None,
                            in_=ts_dram[:, :],
                            in_offset=bass.IndirectOffsetOnAxis(ap=idx_t[:, k:k + 1], axis=0))
                    st_t = sb2.tile([P, Kt * P], f32, name="stf", tag="stf")
                    nc.sync.dma_start(st_t[:], st_fd[:, off * P:(off + Kt) * P])
                    s_t = sb2.tile([P, Kt * P], f32, name="sf", tag="sf")
                    nc.sync.dma_start(s_t[:], s_fd[:, off * P:(off + Kt) * P])
                    eat = sb2.tile([16, Kt * P], f32, name="eat", tag="eat")
                    nc.sync.dma_start(eat[:], ea_td[:, off * P:(off + Kt) * P])
                    xdt = sb2.tile([P, 72], f32, name="xdt", tag="xdt")
                    nc.sync.dma_start(xdt[:], xdp[L][t * P:(t + 1) * P, :])

                    ph = [ps_ph.tile([P, 455], f32, name="ph", tag="ph", space="PSUM")
                          for _ in range(ng)]
                    ppos = ps_ppx.tile([P, Kt * 4], f32, name="ppx", tag="ppx", space="PSUM")
                    for k in range(Kt):
                        nc.tensor.matmul(out=ppos[:, k * 4:(k + 1) * 4],
                                         lhsT=st_t[:, k * P:(k + 1) * P],
                                         rhs=xdt[:, 64:68], start=True, stop=True)
                    # diff / radial (edge-major, batched over all K)
                    dstage = sb.tile([P, Kt * 4], f32, name="dst", tag="dst")
                    d3 = dstage[:].rearrange("p (k c) -> p k c", c=4)
                    p3 = ppos[:].rearrange("p (k c) -> p k c", c=4)
                    g3 = g_t[:].rearrange("p (k c) -> p k c", c=68)
                    nc.vector.tensor_tensor(out=d3[:, :, 0:3], in0=p3[:, :, 0:3],
                                            in1=g3[:, :, 64:67], op=ALU.add)
                    dsq = sb.tile([P, Kt * 4], f32, name="dsq", tag="dsq")
                    q3 = dsq[:].rearrange("p (k c) -> p k c", c=4)
                    nc.vector.tensor_tensor(out=q3[:, :, 0:3], in0=d3[:, :, 0:3],
                                            in1=d3[:, :, 0:3], op=ALU.mult)
                    radst = sb.tile([P, Kt], f32, name="rad", tag="rad")
                    nc.vector.tensor_reduce(out=radst[:].rearrange("p (k o) -> p k o", o=1),
                                            in_=q3[:, :, 0:3], axis=AX.X, op=ALU.add)
                    rwr = sb.tile([P, Kt * 64], f32, name="rwr", tag="rwr")
                    for k in range(Kt):
                        nc.vector.tensor_scalar_mul(rwr[:, k * 64:(k + 1) * 64],
                                                    W['wrrep' + sfx][:],
                                                    radst[:, k:k + 1])
                    for k in range(Kt):
                        gi, c = k // 7, (k % 7) * 64
                        nc.tensor.matmul(out=ph[gi][:, c:c + 64],
                                         lhsT=st_t[:, k * P:(k + 1) * P],
                                         rhs=xdt[:, 0:64], start=True, stop=False)
                        nc.tensor.matmul(out=ph[gi][:, c:c + 64],
                                         lhsT=eat[:, k * P:(k + 1) * P],
                                         rhs=W['wea' + sfx][:], start=False, stop=False)
                        nc.tensor.matmul(out=ph[gi][:, c:c + 64], lhsT=ident[:],
                                         rhs=g_t[:, k * 68:k * 68 + 64],
                                         start=False, stop=False)
                        nc.tensor.matmul(out=ph[gi][:, c:c + 64], lhsT=ident[:],
                                         rhs=rwr[:, k * 64:(k + 1) * 64],
                                         start=False, stop=True)
                    h_st = sb.tile([P, Kt * 64], f32, name="hst", tag="hst")
                    for gi in range(ng):
                        nblk = min(7, Kt - gi * 7)
                        nc.scalar.activation(out=h_st[:, gi * 448: gi * 448 + nblk * 64],
                                             in_=ph[gi][:, 0:nblk * 64], func=AF.Silu)
                    # transpose h -> hT1 (65, K*128) with ones row
                    hT1 = sb.tile([65, Kt * P], f32, name="hT1", tag="hT1")
                    for s in range((Kt + 3) // 4):
                        phT = ps_phT.tile([65, 512], f32, name="phT", tag="phT", space="PSUM")
                        nb = min(4, Kt - s * 4)
                        for j in range(nb):
                            k = s * 4 + j
                            nc.tensor.transpose(out=phT[0:64, j * P:(j + 1) * P],
                                                in_=h_st[:, k * 64:(k + 1) * 64],
                                                identity=ident[:])
                        nc.vector.memset(phT[64:65, 0:nb * P], 1.0)
                        nc.scalar.activation(out=hT1[:, s * 512: s * 512 + nb * P],
                                             in_=phT[:, 0:nb * P], func=AF.Copy)
                    # e1 | s
                    pes = [ps_ph.tile([P, 448], f32, name="ph", tag="ph", space="PSUM")
                           for _ in range(ng)]
                    for k in range(Kt):
                        gi, c = k // 7, (k % 7) * 64
                        nc.tensor.matmul(out=pes[gi][:, c:c + 64],
                                         lhsT=hT1[:, k * P:(k + 1) * P],
                                         rhs=W['we1s' + sfx][:], start=True, stop=True)
                    scat = sb.tile([P, Kt * 68], f32, name="scat", tag="scat")
                    sc3 = scat[:].rearrange("p (k c) -> p k c", c=68)
                    for gi in range(ng):
                        nblk = min(7, Kt - gi * 7)
                        e3 = pes[gi][:, 0:nblk * 64].rearrange("p (k c) -> p k c", c=64)
                        nc.scalar.activation(out=sc3[:, gi * 7: gi * 7 + nblk, 0:64],
                                             in_=e3[:, :, 0:64], func=AF.Silu)
                    # coord gate: s = silu(e1 @ cw + cb) via DVE dot
                    e1m = sb.tile([P, Kt * 64], f32, name="e1m", tag="e1m")
                    for k in range(Kt):
                        nc.vector.tensor_tensor(out=e1m[:, k * 64:(k + 1) * 64],
                                                in0=scat[:, k * 68:k * 68 + 64],
                                                in1=W['cwrep' + sfx][:], op=ALU.mult)
                    sgate = sb.tile([P, Kt], f32, name="sgate", tag="sgate")
                    nc.vector.tensor_reduce(
                        out=sgate[:].rearrange("p (k o) -> p k o", o=1),
                        in_=e1m[:].rearrange("p (k c) -> p k c", c=64),
                        axis=AX.X, op=ALU.add)
                    nc.scalar.activation(out=sgate[:], in_=sgate[:], func=AF.Silu,
                                         bias=W['cbrep' + sfx][:, 0:1])
                    s3 = sgate[:].rearrange("p (k o) -> p k o", o=1)
                    for c in range(3):
                        nc.vector.tensor_tensor(out=sc3[:, :, 65 + c:66 + c],
                                                in0=d3[:, :, c:c + 1],
                                                in1=s3[:, :, 0:1], op=ALU.mult)
                    nc.vector.memset(sc3[:, :, 64:65], 1.0)
                    pagg = ps_pagg.tile([P, 68], f32, name="pagg", tag="pagg", space="PSUM")
                    for k in range(Kt):
                        nc.tensor.matmul(out=pagg[:], lhsT=s_t[:, k * P:(k + 1) * P],
                                         rhs=scat[:, k * 68:(k + 1) * 68],
                                         start=(k == 0), stop=(k == Kt - 1))
                    # ---- node stage ----
                    eagg = sb.tile([P, 68], f32, name="eagg", tag="eagg")
                    nc.vector.tensor_copy(out=eagg[:], in_=pagg[:])
                    if L == 0 and t == 0:
                        nc.sync.dma_start(dbg_h[:, 0:Kt * 64], h_st[:])
                        nc.sync.dma_start(dbg_sc[:, 0:Kt * 68], scat[:])
                        nc.sync.dma_start(dbg_ea[:, :], eagg[:])
                        nc.sync.dma_start(dbg_rad[:, 0:Kt], radst[:])
                        nc.sync.dma_start(dbg_g[:, 0:Kt * 68], g_t[:])
                    deg1 = sb.tile([P, 1], f32, name="deg", tag="deg")
                    nc.vector.tensor_scalar_max(deg1[:], eagg[:, 64:65], 1.0)
                    inv = sb.tile([P, 1], f32, name="inv", tag="inv")
                    nc.vector.reciprocal(out=inv[:], in_=deg1[:])
                    posn = sb.tile([P, 4], f32, name="posn", tag="posn")
                    nc.vector.tensor_scalar_mul(posn[:, 0:3], eagg[:, 65:68], inv[:, 0:1])
                    nc.vector.tensor_tensor(out=posn[:, 0:3], in0=posn[:, 0:3],
                                            in1=pos_own[:, t * 4:t * 4 + 3], op=ALU.add)
                    nc.vector.memset(posn[:, 3:4], 0.0)
                    nc.vector.tensor_copy(out=posn_all[:, t * 4:(t + 1) * 4], in_=posn[:])
                    pet = ps_phT.tile([65, 512], f32, name="phT", tag="phT", space="PSUM")
                    nc.tensor.transpose(out=pet[0:64, 0:P], in_=eagg[:, 0:64],
                                        identity=ident[:])
                    eaT = sb.tile([64, P], f32, name="eaT", tag="eaT")
                    nc.scalar.activation(out=eaT[:], in_=pet[0:64, 0:P], func=AF.Copy)
                    pn1 = ps_ph.tile([64, P], f32, name="ph", tag="ph", space="PSUM")
                    nc.tensor.matmul(out=pn1[:], lhsT=W['wn1x' + sfx][:],
                                     rhs=xin_T[:, t * P:(t + 1) * P], start=True, stop=False)
                    nc.tensor.matmul(out=pn1[:], lhsT=W['wn1a' + sfx][:], rhs=eaT[:],
                                     start=False, stop=False)
                    nc.tensor.matmul(out=pn1[:], lhsT=W['nb1_' + sfx][:], rhs=ones_row[:],
                                     start=False, stop=True)
                    zst = sb.tile([65, P], f32, name="zst", tag="zst")
                    nc.scalar.activation(out=zst[0:64, :], in_=pn1[:], func=AF.Silu)
                    nc.vector.memset(zst[64:65, :], 1.0)
                    px1 = ps_ph.tile([P, P], f32, name="ph", tag="ph", space="PSUM")
                    nc.tensor.matmul(out=px1[:], lhsT=W['wn2b' + sfx][:], rhs=zst[:],
                                     start=True, stop=True)
                    nc.scalar.activation(out=xout_T[:, t * P:(t + 1) * P], in_=px1[:],
                                         func=AF.Copy)
                    if L == 0:
                        pp = ps_ph.tile([P, 64], f32, name="ph", tag="ph", space="PSUM")
                        nc.tensor.matmul(out=pp[:], lhsT=xout_T[:, t * P:(t + 1) * P],
                                         rhs=W['wproj1'][:, 64:128], start=True, stop=True)
                        tst = sb.tile([P, 68], f32, name="tst", tag="tst")
                        nc.vector.tensor_copy(out=tst[:, 0:64], in_=pp[:])
                        nc.vector.tensor_scalar_mul(tst[:, 64:67], posn[:, 0:3], -1.0)
                        nc.vector.memset(tst[:, 67:68], 0.0)
                        writes.append(nc.sync.dma_start(ts1sh[t * P:(t + 1) * P, :], tst[:]))
                        pp2 = ps_ph.tile([P, 64], f32, name="ph", tag="ph", space="PSUM")
                        nc.tensor.matmul(out=pp2[:], lhsT=xout_T[:, t * P:(t + 1) * P],
                                         rhs=W['wproj1'][:, 0:64], start=True, stop=True)
                        xds = sb.tile([P, 72], f32, name="xds", tag="xds")
                        nc.vector.tensor_copy(out=xds[:, 0:64], in_=pp2[:])
                        nc.vector.tensor_copy(out=xds[:, 64:68], in_=posn[:])
                        nc.vector.memset(xds[:, 68:72], 0.0)
                        writes.append(nc.sync.dma_start(xdp[1][t * P:(t + 1) * P, :], xds[:]))
                    else:
                        bpt = sb.tile([P, 64], f32, name="bpt", tag="bpt")
                        nc.sync.dma_start(bpt[:], bpool_d[:, t * 64:(t + 1) * 64])
                        pxn = ps_ph.tile([P, P], f32, name="ph", tag="ph", space="PSUM")
                        nc.tensor.transpose(out=pxn[:], in_=xout_T[:, t * P:(t + 1) * P],
                                            identity=ident[:])
                        x2n = sb.tile([P, P], f32, name="x2n", tag="x2n")
                        nc.scalar.activation(out=x2n[:], in_=pxn[:], func=AF.Copy)
                        nc.tensor.matmul(out=ppool_t[:], lhsT=bpt[:], rhs=x2n[:],
                                         start=(t == 0), stop=(t == T - 1))
                return writes

            # layer 0
            tc.strict_bb_all_engine_barrier()
            l0_writes = layer(0, ts0, xT_a, xT_b)
            # allgather ts1
            tc.strict_bb_all_engine_barrier()
            cc1 = nc.gpsimd.collective_compute(
                "AllGather", ALU.bypass, replica_groups=[list(range(NC))],
                ins=[ts1sh.ap().opt()], outs=[ts1.ap().opt()])
            tc.strict_bb_all_engine_barrier()
            # layer 1 (+ pooling accumulation)
            ppool_t = ps_ppool.tile([G_, P], f32, name="ppool", tag="ppool", space="PSUM")
            layer(1, ts1, xT_b, xT_a)
            # pooling tail
            gss = sb.tile([G_, P], f32, name="gss", tag="gss")
            nc.vector.tensor_copy(out=gss[:], in_=ppool_t[:])
            nc.sync.dma_start(gs_in[:, :], gss[:])
            tc.strict_bb_all_engine_barrier()
            cc2 = nc.gpsimd.collective_compute(
                "AllReduce", ALU.add, replica_groups=[list(range(NC))],
                ins=[gs_in.ap().opt()], outs=[gs_out.ap().opt()])
            tc.strict_bb_all_engine_barrier()
            gsr = sb.tile([G_, P], f32, name="gsr", tag="gsr")
            nc.sync.dma_start(gsr[:], gs_out[:, :])
            gm = sb.tile([G_, P], f32, name="gm", tag="gm")
            nc.vector.tensor_scalar_mul(gm[:], gsr[:], invcnt_t[:, 0:1])
            gr = sb.tile([G_, P], f32, name="gr", tag="gr")
            nc.scalar.activation(out=gr[:], in_=gm[:], func=AF.Relu)
            pgt = ps_ph.tile([P, G_], f32, name="ph", tag="ph", space="PSUM")
            nc.tensor.transpose(out=pgt[:, 0:G_], in_=gr[:], identity=ident[0:G_, 0:G_])
            gT = sb.tile([P, G_], f32, name="gT", tag="gT")
            nc.scalar.activation(out=gT[:], in_=pgt[:, 0:G_], func=AF.Copy)
            po1 = ps_ph.tile([P, G_], f32, name="ph", tag="ph", space="PSUM")
            nc.tensor.matmul(out=po1[:], lhsT=W['wo1'][:], rhs=gT[:], start=True, stop=False)
            nc.tensor.matmul(out=po1[:], lhsT=W['wo1b'][:], rhs=ones_row[:, 0:G_],
                             start=False, stop=True)
            r1 = sb.tile([P, G_], f32, name="r1", tag="r1")
            nc.scalar.activation(out=r1[:], in_=po1[:], func=AF.Relu)
            po2 = ps_ph.tile([32, G_], f32, name="ph", tag="ph", space="PSUM")
            nc.tensor.matmul(out=po2[:], lhsT=W['wo2'][:], rhs=r1[:], start=True, stop=False)
            nc.tensor.matmul(out=po2[:], lhsT=W['wo2b'][:], rhs=ones_row[:, 0:G_],
                             start=False, stop=True)
            o2 = sb.tile([32, G_], f32, name="o2", tag="o2")
            nc.scalar.activation(out=o2[:], in_=po2[:], func=AF.Copy)
            pot = ps_ph.tile([G_, 32], f32, name="ph", tag="ph", space="PSUM")
            nc.tensor.transpose(out=pot[0:G_, 0:32], in_=o2[:], identity=ident[0:32, 0:32])
            oT = sb.tile([G_, 32], f32, name="oT", tag="oT")
            nc.scalar.activation(out=oT[:], in_=pot[0:G_, 0:32], func=AF.Copy)
            nc.sync.dma_start(out_ext[:, :], oT[:])

    return nc


def run(inputs, n_tiles_per_core, trace=False):
    st = host_prep(inputs, n_tiles_per_core)
    w = host_weights(inputs)
    N = inputs['x'].shape[0]
    NPAD, SH = st['NPAD'], st['SH']
    xf = np.zeros((NPAD, P), np.float32)
    xf[:N] = np.asarray(inputs['x'], np.float32)
    pf = np.zeros((NPAD, 4), np.float32)
    pf[:N, 0:3] = np.asarray(inputs['pos'], np.float32)
    nc = build(st)
    in_maps = []
    for c in range(NC):
        m = dict(x_full=xf, pos_full=pf,
                 x_own=np.ascontiguousarray(xf[c * SH:(c + 1) * SH]),
                 pos_own_in=np.ascontiguousarray(pf[c * SH:(c + 1) * SH]),
                 src_pm=st['src_pm'][c], ea_t=st['ea_t'][c],
                 s_full=st['s_full'][c], st_full=st['st_full'][c],
                 bpool=st['bpool'][c], invcnt=st['invcnt'])
        m.update(w)
        in_maps.append(m)
    res = bass_utils.run_bass_kernel_spmd(nc, in_maps, core_ids=list(range(NC)),
                                          trace=trace)
    return res


def kernel(**inputs):
    n_tiles = math.ceil(inputs['x'].shape[0] / (P * NC))
    res = run(inputs, n_tiles)
    return res.results[0]['out']



# revision 7
# speedup vs baseline: 3.0152x; 3.0152x over previous
"""EGNN (2-layer, graph pooling) Trainium2 SPMD kernel over 8 NeuronCores.

Edges are dst-sorted and sharded by destination-node range. All matmul
operands are bf16 (fp32 PSUM accumulate). Layer 0 needs no gathers: the host
ships raw x[src] feature blocks (transposed) per edge slot and the device
projects them per block. Layer 1 gathers [proj1|−pos1] rows from an
AllGathered node table via per-block indirect DMA. Scatter/broadcast use
host-built bf16 one-hot matmuls on the TensorEngine; graph pooling uses
one-hot matmuls and a final AllReduce. Walrus in this environment accepts one
sync-wait per instruction, so a JSON-level pass splits multi-wait
instructions onto NoOp carriers.
"""
import sys
sys.path.insert(0, '/opt/trn_rl_repo')
import concourse.tile as tile_mod
from concourse.vector_clock import ScopedClock


def _patched_drain_and_barrier(self, tick_clock, wait_clock):
    nc = self.nc
    probe = nc.sync.nop(nofuse=True)
    wait_clock.add_sem_waits(probe.ins, ScopedClock({None: tick_clock.global_clock}))
    waits = list(probe.ins.sync_info.on_wait)
    probe.ins.sync_info.on_wait = []
    import concourse.mybir as mybir
    for w in waits:
        carrier = nc.sync.nop(nofuse=True)
        if carrier.ins.sync_info is None:
            carrier.ins.sync_info = mybir.SyncInfo(on_wait=[], on_update=[])
        carrier.ins.sync_info.on_wait = [w]
    nc.sync.drain()

    nc.all_engine_barrier()
    assert self.sems is not None
    popped = nc._tile_sem_poison_stack.pop()
    assert popped is self._sem_poison
    nc.clear_and_free_semaphores(list(self.sems.allocated().values()))
    nc.all_engine_barrier()


def apply_patch():
    tile_mod.TileContext._drain_and_barrier = _patched_drain_and_barrier


def _legalize_waits_json(mod: dict) -> dict:
    """Walrus in this env accepts at most ONE sync wait per instruction.
    Split extra waits onto same-engine NoOp carriers inserted just before."""
    n_new = [0]
    for fn in mod.get('functions', []):
        for blk in fn.get('blocks', []):
            insts = blk.get('instructions', [])
            out = []
            for inst in insts:
                si = inst.get('sync_info') or {}
                waits = si.get('on_wait') or []
                if len(waits) > 1:
                    eng = inst.get('engine')
                    for w in waits[:-1]:
                        n_new[0] += 1
                        out.append({
                            'debug': inst.get('debug', 0),
                            'engine': eng, 'ins': [], 'outs': [],
                            'name': 'I-waitfix-%d' % n_new[0],
                            'opcode': 'NoOp',
                            'sync_info': {'on_update': [], 'on_wait': [w]},
                        })
                    si['on_wait'] = [waits[-1]]
                out.append(inst)
            blk['instructions'] = out
    return mod


def apply_json_patch():
    import orjson
    import concourse.bass as bass_mod
    orig = bass_mod.Bass.to_json_bytes
    def to_json_bytes(self):
        raw = orig(self)
        mod = orjson.loads(raw)
        mod = _legalize_waits_json(mod)
        return orjson.dumps(mod)
    bass_mod.Bass.to_json_bytes = to_json_bytes


import math
import numpy as np
import ml_dtypes
import concourse.bass as bass
import concourse.mybir as mybir
from concourse.tile import TileContext
from concourse import bass_utils
from concourse.masks import make_identity
apply_patch(); apply_json_patch()

f32 = mybir.dt.float32
bf16 = mybir.dt.bfloat16
i32 = mybir.dt.int32
AF = mybir.ActivationFunctionType
ALU = mybir.AluOpType
AX = mybir.AxisListType
P = 128
NC = 8
GRP = 7          # edge blocks per psum group (7*68 = 476 <= 512)
BF = ml_dtypes.bfloat16


def host_prep(inputs, n_tiles_per_core):
    SH = n_tiles_per_core * P
    NPAD = SH * NC
    T = n_tiles_per_core
    src = np.asarray(inputs['edge_index'][0], np.int64)
    dst = np.asarray(inputs['edge_index'][1], np.int64)
    ea = np.asarray(inputs['edge_attr'], np.float32)
    order = np.argsort(dst, kind='stable')
    src, dst, ea = src[order], dst[order], ea[order]
    core_of = dst // SH
    tile_of = (dst % SH) // P

    counts = np.zeros((NC, T), np.int64)
    for c in range(NC):
        m = core_of == c
        tl, cn = np.unique(tile_of[m], return_counts=True)
        counts[c, tl] = cn
    K = np.maximum(1, np.ceil(counts / P).astype(np.int64).max(axis=0))
    offs = np.concatenate([[0], np.cumsum(K)]).astype(np.int64)
    TOT = int(offs[-1])

    xbf = np.zeros((NPAD, P), BF)
    xbf[:len(inputs['x'])] = np.asarray(inputs['x'], np.float32).astype(BF)

    pf = np.zeros((NPAD, 4), np.float32)
    pf[:len(inputs['pos']), 0:3] = np.asarray(inputs['pos'], np.float32)
    pbf = pf.astype(BF)

    src_pm = np.zeros((NC, P, TOT), np.int32)
    ea_t = np.zeros((NC, 16, TOT * P), BF)
    s_full = np.zeros((NC, P, TOT * P), BF)
    st_full = np.zeros((NC, P, TOT * P), BF)
    xsT = np.zeros((NC, P, TOT * P), BF)          # x[src].T per block
    nsp = np.zeros((NC, P, TOT * 4), np.float32)  # -pos[src] per edge slot
    for c in range(NC):
        m = core_of == c
        s_c, d_c, e_c, t_c = src[m], dst[m], ea[m], tile_of[m]
        for t in range(T):
            mt = t_c == t
            sc, dc, ec = s_c[mt], d_c[mt], e_c[mt]
            nloc = ((dc - c * SH) - t * P).astype(np.int64)
            n_e = len(sc)
            for k in range(int(K[t])):
                blk = int(offs[t]) + k
                lo, hi = k * P, min((k + 1) * P, n_e)
                cnt = max(0, hi - lo)
                if cnt > 0:
                    src_pm[c, :cnt, blk] = sc[lo:hi]
                    ea_t[c, :, blk * P: blk * P + cnt] = ec[lo:hi].T.astype(BF)
                    oh = np.zeros((P, P), np.float32)
                    oh[np.arange(cnt), nloc[lo:hi]] = 1.0
                    s_full[c, :, blk * P:(blk + 1) * P] = oh.astype(BF)
                    st_full[c, :, blk * P:(blk + 1) * P] = oh.T.astype(BF)
                    xsT[c, :, blk * P: blk * P + cnt] = xbf[sc[lo:hi]].T
                    npos = -pf[sc[lo:hi]]
                    nsp[c, :cnt, blk * 4:(blk + 1) * 4] = npos
    # pooling one-hots + counts
    batch = np.asarray(inputs['batch'], np.int64)
    G_ = 64
    bpool = np.zeros((NC, P, T * 64), BF)
    N = len(batch)
    for c in range(NC):
        for t in range(T):
            for p in range(P):
                n = c * SH + t * P + p
                if n < N:
                    bpool[c, p, t * 64 + int(batch[n])] = 1.0
    cnts = np.bincount(batch, minlength=G_).astype(np.float32)
    invcnt = (1.0 / np.maximum(cnts, 1.0)).reshape(G_, 1)
    return dict(SH=SH, NPAD=NPAD, T=T, K=[int(k) for k in K],
                offs=[int(o) for o in offs], TOT=TOT, src_pm=src_pm,
                ea_t=ea_t, s_full=s_full, st_full=st_full, xsT=xsT, nsp=nsp,
                bpool=bpool, invcnt=invcnt, xbf=xbf, pbf=pbf)


def host_weights(inputs):
    w = {}
    for L in range(2):
        mw = np.asarray(inputs[f'l{L}_mlp_w'], np.float32)
        # dst-proj (rows 0:in_c) and src-proj (rows in_c:2in_c)
        w[f'wdst{L}'] = mw[0:128].astype(BF)
        w[f'wsrc{L}'] = mw[128:256].astype(BF)
        w[f'wea{L}'] = np.ascontiguousarray(mw[256:272]).astype(BF)
        w[f'wrrep{L}'] = np.tile(mw[272:273], (P, 1)).astype(np.float32)
        ew = np.asarray(inputs[f'l{L}_edge_w'], np.float32)
        eb = np.asarray(inputs[f'l{L}_edge_b'], np.float32)
        cw = np.asarray(inputs[f'l{L}_coord_w'], np.float32)
        cb = np.asarray(inputs[f'l{L}_coord_b'], np.float32)
        we1s = np.zeros((65, 64), np.float32)
        we1s[0:64, 0:64] = ew; we1s[64, 0:64] = eb
        w[f'we1s{L}'] = we1s.astype(BF)
        w[f'cwrep{L}'] = np.tile(cw[:, 0][None, :], (P, 1)).astype(BF)
        w[f'cbrep{L}'] = np.full((P, 1), cb[0], np.float32)
        n1 = np.asarray(inputs[f'l{L}_node_w1'], np.float32)
        w[f'wn1x{L}'] = np.ascontiguousarray(n1[0:128]).astype(BF)
        w[f'wn1a{L}'] = np.ascontiguousarray(n1[128:192]).astype(BF)
        w[f'nb1_{L}'] = np.asarray(inputs[f'l{L}_node_b1'], np.float32)[None, :].astype(BF)
        w[f'wn2b{L}'] = np.concatenate(
            [np.asarray(inputs[f'l{L}_node_w2'], np.float32),
             np.asarray(inputs[f'l{L}_node_b2'], np.float32)[None, :]], 0).astype(BF)
    w['wo1'] = np.asarray(inputs['out_w1'], np.float32).astype(BF)
    w['wo1b'] = np.asarray(inputs['out_b1'], np.float32)[None, :].astype(BF)
    w['wo2'] = np.asarray(inputs['out_w2'], np.float32).astype(BF)
    w['wo2b'] = np.asarray(inputs['out_b2'], np.float32)[None, :].astype(BF)
    return w


WSHAPES = dict(wdst0=(P, 64), wdst1=(P, 64), wsrc0=(P, 64), wsrc1=(P, 64),
               wea0=(16, 64), wea1=(16, 64),
               we1s0=(65, 64), we1s1=(65, 64),
               wn1x0=(P, 64), wn1x1=(P, 64), wn1a0=(64, 64), wn1a1=(64, 64),
               nb1_0=(1, 64), nb1_1=(1, 64), wn2b0=(65, P), wn2b1=(65, P),
               wo1=(P, P), wo1b=(1, P), wo2=(P, 32), wo2b=(1, 32))
WSHAPES_F32 = dict(wrrep0=(P, 64), wrrep1=(P, 64),
                   cbrep0=(P, 1), cbrep1=(P, 1))
WSHAPES_CW = dict(cwrep0=(P, 64), cwrep1=(P, 64))


def build(st):
    SH, NPAD, T, K, offs, TOT = (st['SH'], st['NPAD'], st['T'], st['K'],
                                 st['offs'], st['TOT'])
    G_, OUT = 64, 32
    KMAX = max(K)

    nc = bass.Bass("TRN2")
    dram = {}
    def din(name, shape, dt=bf16):
        dram[name] = nc.dram_tensor(name, shape, dt, kind="ExternalInput")
        return dram[name]

    x_own = din('x_own', (SH, P))
    pos_own_in = din('pos_own_in', (SH, 4))
    src_pm = din('src_pm', (P, TOT), i32)
    ea_td = din('ea_t', (16, TOT * P))
    s_fd = din('s_full', (P, TOT * P))
    st_fd = din('st_full', (P, TOT * P))
    xsT_d = din('xsT', (P, TOT * P))
    nsp_d = din('nsp', (P, TOT * 4), f32)
    bpool_d = din('bpool', (P, T * 64))
    invcnt_d = din('invcnt', (G_, 1), f32)
    for n, shp in WSHAPES.items():
        din(n, shp)
    for n, shp in WSHAPES_F32.items():
        din(n, shp, f32)
    for n, shp in WSHAPES_CW.items():
        din(n, shp)
    out_ext = nc.dram_tensor('out', (G_, OUT), f32, kind="ExternalOutput")
    dbg_h = nc.dram_tensor('dbg_h', (P, K[0] * 64), f32, kind="ExternalOutput")
    dbg_eagg = nc.dram_tensor('dbg_eagg', (P, 68), f32, kind="ExternalOutput")
    dbg_scat = nc.dram_tensor('dbg_scat', (P, K[0] * 68), f32, kind="ExternalOutput")
    dbg_x1 = nc.dram_tensor('dbg_x1', (P, P), f32, kind="ExternalOutput")
    dbg_tst = nc.dram_tensor('dbg_tst', (P, 68), f32, kind="ExternalOutput")
    dbg_eagg1 = nc.dram_tensor('dbg_eagg1', (P, 68), f32, kind="ExternalOutput")
    dbg_g1 = nc.dram_tensor('dbg_g1', (P, K[0] * 68), f32, kind="ExternalOutput")

    ts1sh = nc.dram_tensor('ts1sh', (SH, 68), bf16)
    ts1 = nc.dram_tensor('ts1', (NPAD, 68), bf16, addr_space="Shared")
    xdp = [nc.dram_tensor('xdp0', (SH, 72), bf16),
           nc.dram_tensor('xdp1', (SH, 72), bf16)]
    gs_in = nc.dram_tensor('gs_in', (G_, P), f32)
    gs_out = nc.dram_tensor('gs_out', (G_, P), f32, addr_space="Shared")

    with TileContext(nc) as tc:
        with (tc.tile_pool(name="pers", bufs=1) as pers,
              tc.tile_pool(name="sb", bufs=2) as sb,
              tc.tile_pool(name="sb2", bufs=2) as sb2,
              tc.tile_pool(name="ph", bufs=4, space="PSUM") as ps_ph,
              tc.tile_pool(name="pagg", bufs=1, space="PSUM") as ps_pagg,
              tc.tile_pool(name="phT", bufs=1, space="PSUM") as ps_phT,
              tc.tile_pool(name="ppool", bufs=1, space="PSUM") as ps_ppool):

            ident = pers.tile([P, P], bf16, name="ident", tag="ident")
            make_identity(nc, ident[:])
            ones_row = pers.tile([1, P], bf16, name="ones_row", tag="ones_row")
            nc.vector.memset(ones_row[:], 1.0)
            xT_a = pers.tile([P, SH], bf16, name="xT_a", tag="xT_a")
            xT_b = pers.tile([P, SH], bf16, name="xT_b", tag="xT_b")
            pos_own = pers.tile([P, T * 4], f32, name="pos_own", tag="pos_own")
            posn_all = pers.tile([P, T * 4], f32, name="posn_all", tag="posn_all")
            invcnt_t = pers.tile([G_, 1], f32, name="invc", tag="invc")
            nc.sync.dma_start(invcnt_t[:], invcnt_d[:, :])
            # persistent transposed-h buffers with a preset ones row (row 64)
            hT1_ab = [pers.tile([65, KMAX * P], bf16, name=f"hT1{i}", tag=f"hT1{i}")
                      for i in range(2)]
            for hb in hT1_ab:
                nc.vector.memset(hb[64:65, :], 1.0)
            W = {}
            for n, shp in WSHAPES.items():
                W[n] = pers.tile(list(shp), bf16, name="w_" + n, tag="w_" + n)
                nc.sync.dma_start(W[n][:], dram[n][:, :])
            for n, shp in WSHAPES_F32.items():
                W[n] = pers.tile(list(shp), f32, name="w_" + n, tag="w_" + n)
                nc.sync.dma_start(W[n][:], dram[n][:, :])
            for n, shp in WSHAPES_CW.items():
                W[n] = pers.tile(list(shp), bf16, name="w_" + n, tag="w_" + n)
                nc.sync.dma_start(W[n][:], dram[n][:, :])

            # ---------------- stage A: own-shard prep ----------
            for t in range(T):
                xt = sb.tile([P, P], bf16, name="ax", tag="ax")
                nc.sync.dma_start(xt[:], x_own[t * P:(t + 1) * P, :])
                pt = sb.tile([P, 4], bf16, name="ap", tag="ap")
                nc.sync.dma_start(pt[:], pos_own_in[t * P:(t + 1) * P, :])
                nc.vector.tensor_copy(out=pos_own[:, t * 4:(t + 1) * 4], in_=pt[:])
                pxt = ps_ph.tile([P, P], bf16, name="ph", tag="ph", space="PSUM")
                nc.tensor.transpose(out=pxt[:], in_=xt[:], identity=ident[:])
                nc.scalar.activation(out=xT_a[:, t * P:(t + 1) * P], in_=pxt[:], func=AF.Copy)
                pp = ps_ph.tile([P, 64], f32, name="ph", tag="ph", space="PSUM")
                nc.tensor.matmul(out=pp[:], lhsT=xT_a[:, t * P:(t + 1) * P],
                                 rhs=W['wdst0'][:], start=True, stop=True)
                xds = sb.tile([P, 72], bf16, name="xds", tag="xds")
                nc.vector.tensor_copy(out=xds[:, 0:64], in_=pp[:])
                nc.vector.tensor_copy(out=xds[:, 64:68], in_=pt[:])
                nc.vector.memset(xds[:, 68:72], 0.0)
                nc.sync.dma_start(xdp[0][t * P:(t + 1) * P, :], xds[:])

            # ---------------- edge + node stage, per layer --------------
            def layer(L, xin_T, xout_T):
                sfx = str(L)
                for t in range(T):
                    Kt = K[t]
                    off = offs[t]
                    ng = (Kt + GRP - 1) // GRP
                    hT1 = hT1_ab[t % 2]
                    st_t = sb2.tile([P, Kt * P], bf16, name="stf", tag="stf")
                    nc.sync.dma_start(st_t[:], st_fd[:, off * P:(off + Kt) * P])
                    s_t = sb2.tile([P, Kt * P], bf16, name="sf", tag="sf")
                    nc.sync.dma_start(s_t[:], s_fd[:, off * P:(off + Kt) * P])
                    eat = sb2.tile([16, Kt * P], bf16, name="eat", tag="eat")
                    nc.sync.dma_start(eat[:], ea_td[:, off * P:(off + Kt) * P])
                    xdt = sb2.tile([P, 72], bf16, name="xdt", tag="xdt")
                    nc.sync.dma_start(xdt[:], xdp[L][t * P:(t + 1) * P, :])
                    if L == 0:
                        xsrc = sb2.tile([P, Kt * P], bf16, name="xsrc", tag="xsrc")
                        nc.sync.dma_start(xsrc[:], xsT_d[:, off * P:(off + Kt) * P])
                        nspt = sb2.tile([P, Kt * 4], f32, name="nsp", tag="nsp")
                        nc.sync.dma_start(nspt[:], nsp_d[:, off * 4:(off + Kt) * 4])
                    else:
                        idx_t = sb2.tile([P, Kt], i32, name="idx", tag="idx")
                        nc.sync.dma_start(idx_t[:], src_pm[:, off:off + Kt])
                        g_t = sb2.tile([P, Kt * 68], bf16, name="G", tag="G")
                        for k in range(Kt):
                            nc.gpsimd.indirect_dma_start(
                                out=g_t[:, k * 68:(k + 1) * 68], out_offset=None,
                                in_=ts1[:, :],
                                in_offset=bass.IndirectOffsetOnAxis(ap=idx_t[:, k:k + 1], axis=0))

                    # ---- z psum assembly (68-wide slices per block) ----
                    ph = [ps_ph.tile([P, GRP * 68], f32, name="ph", tag="ph", space="PSUM")
                          for _ in range(ng)]
                    for k in range(Kt):
                        gi, c = k // GRP, (k % GRP) * 68
                        nc.tensor.matmul(out=ph[gi][:, c:c + 68],
                                         lhsT=st_t[:, k * P:(k + 1) * P],
                                         rhs=xdt[:, 0:68], start=True, stop=False)
                        if L == 0:
                            nc.tensor.matmul(out=ph[gi][:, c:c + 64],
                                             lhsT=xsrc[:, k * P:(k + 1) * P],
                                             rhs=W['wsrc0'][:],
                                             start=False, stop=False)
                        else:
                            nc.tensor.matmul(out=ph[gi][:, c:c + 68],
                                             lhsT=ident[:],
                                             rhs=g_t[:, k * 68:(k + 1) * 68],
                                             start=False, stop=False)
                        nc.tensor.matmul(out=ph[gi][:, c:c + 64],
                                         lhsT=eat[:, k * P:(k + 1) * P],
                                         rhs=W['wea' + sfx][:], start=False,
                                         stop=True)

                    # ---- diff / radial (DVE) ----
                    dstage = sb.tile([P, Kt * 4], f32, name="dst", tag="dst")
                    d3 = dstage[:].rearrange("p (k c) -> p k c", c=4)
                    for gi in range(ng):
                        nblk = min(GRP, Kt - gi * GRP)
                        p3 = ph[gi][:].rearrange("p (k c) -> p k c", c=68)
                        if L == 0:
                            n3 = nspt[:].rearrange("p (k c) -> p k c", c=4)
                            nc.vector.tensor_tensor(
                                out=d3[:, gi * GRP:gi * GRP + nblk, 0:3],
                                in0=p3[:, 0:nblk, 64:67],
                                in1=n3[:, gi * GRP:gi * GRP + nblk, 0:3], op=ALU.add)
                        else:
                            nc.vector.tensor_copy(
                                out=d3[:, gi * GRP:gi * GRP + nblk, 0:3],
                                in_=p3[:, 0:nblk, 64:67])
                    dsq = sb.tile([P, Kt * 4], f32, name="dsq", tag="dsq")
                    q3 = dsq[:].rearrange("p (k c) -> p k c", c=4)
                    nc.vector.tensor_tensor(out=q3[:, :, 0:3], in0=d3[:, :, 0:3],
                                            in1=d3[:, :, 0:3], op=ALU.mult)
                    radst = sb.tile([P, Kt], f32, name="rad", tag="rad")
                    nc.vector.tensor_reduce(out=radst[:].rearrange("p (k o) -> p k o", o=1),
                                            in_=q3[:, :, 0:3], axis=AX.X, op=ALU.add)
                    # ---- h = silu(z + radial*wr) ----
                    h_st = sb.tile([P, Kt * 64], bf16, name="hst", tag="hst")
                    for gi in range(ng):
                        nblk = min(GRP, Kt - gi * GRP)
                        rwr = sb.tile([P, GRP * 64], f32, name="rwr", tag="rwr")
                        for j in range(nblk):
                            k = gi * GRP + j
                            nc.vector.tensor_scalar_mul(rwr[:, j * 64:(j + 1) * 64],
                                                        W['wrrep' + sfx][:],
                                                        radst[:, k:k + 1])
                        p3 = ph[gi][:].rearrange("p (k c) -> p k c", c=68)
                        hpre = sb.tile([P, GRP * 64], f32, name="hpre", tag="hpre")
                        hp3 = hpre[:].rearrange("p (k c) -> p k c", c=64)
                        r3 = rwr[:].rearrange("p (k c) -> p k c", c=64)
                        nc.vector.tensor_tensor(out=hp3[:, 0:nblk, :],
                                                in0=p3[:, 0:nblk, 0:64],
                                                in1=r3[:, 0:nblk, :], op=ALU.add)
                        nc.scalar.activation(out=h_st[:, gi * GRP * 64: (gi * GRP + nblk) * 64],
                                             in_=hpre[:, 0:nblk * 64], func=AF.Silu)
                    # ---- transpose h -> hT1 (65, K*128), row 64 preset to 1 ----
                    for s in range((Kt + 3) // 4):
                        phT = ps_phT.tile([64, 512], bf16, name="phT", tag="phT", space="PSUM")
                        nb = min(4, Kt - s * 4)
                        for j in range(nb):
                            k = s * 4 + j
                            nc.tensor.transpose(out=phT[0:64, j * P:(j + 1) * P],
                                                in_=h_st[:, k * 64:(k + 1) * 64],
                                                identity=ident[:])
                        nc.vector.tensor_copy(out=hT1[0:64, s * 512: s * 512 + nb * P],
                                              in_=phT[0:64, 0:nb * P])
                    # ---- e1 ----
                    pes = [ps_ph.tile([P, GRP * 64], f32, name="ph", tag="ph", space="PSUM")
                           for _ in range(ng)]
                    for k in range(Kt):
                        gi, c = k // GRP, (k % GRP) * 64
                        nc.tensor.matmul(out=pes[gi][:, c:c + 64],
                                         lhsT=hT1[:, k * P:(k + 1) * P],
                                         rhs=W['we1s' + sfx][:], start=True, stop=True)
                    scat = sb.tile([P, Kt * 68], bf16, name="scat", tag="scat")
                    sc3 = scat[:].rearrange("p (k c) -> p k c", c=68)
                    for gi in range(ng):
                        nblk = min(GRP, Kt - gi * GRP)
                        e3 = pes[gi][:, 0:nblk * 64].rearrange("p (k c) -> p k c", c=64)
                        nc.scalar.activation(out=sc3[:, gi * GRP: gi * GRP + nblk, 0:64],
                                             in_=e3[:, :, 0:64], func=AF.Silu)
                    # ---- coord gate ----
                    e1m = sb.tile([P, Kt * 64], f32, name="e1m", tag="e1m")
                    m3 = e1m[:].rearrange("p (k c) -> p k c", c=64)
                    for k in range(Kt):
                        nc.vector.tensor_tensor(
                            out=e1m[:, k * 64:(k + 1) * 64],
                            in0=scat[:, k * 68:k * 68 + 64],
                            in1=W['cwrep' + sfx][:], op=ALU.mult)
                    sgate = sb.tile([P, Kt], f32, name="sgate", tag="sgate")
                    nc.vector.tensor_reduce(
                        out=sgate[:].rearrange("p (k o) -> p k o", o=1),
                        in_=m3[:, :, :], axis=AX.X, op=ALU.add)
                    nc.scalar.activation(out=sgate[:], in_=sgate[:], func=AF.Silu,
                                         bias=W['cbrep' + sfx][:, 0:1])
                    s3 = sgate[:].rearrange("p (k o) -> p k o", o=1)
                    for cc in range(3):
                        nc.vector.tensor_tensor(out=sc3[:, :, 65 + cc:66 + cc],
                                                in0=d3[:, :, cc:cc + 1],
                                                in1=s3[:, :, 0:1], op=ALU.mult)
                    nc.vector.memset(sc3[:, :, 64:65], 1.0)
                    # ---- scatter to dst nodes ----
                    pagg = ps_pagg.tile([P, 68], f32, name="pagg", tag="pagg", space="PSUM")
                    for k in range(Kt):
                        nc.tensor.matmul(out=pagg[:], lhsT=s_t[:, k * P:(k + 1) * P],
                                         rhs=scat[:, k * 68:(k + 1) * 68],
                                         start=(k == 0), stop=(k == Kt - 1))
                    # ---- node stage ----
                    eagg = sb.tile([P, 68], f32, name="eagg", tag="eagg")
                    nc.vector.tensor_copy(out=eagg[:], in_=pagg[:])
                    if t == 0:
                        nc.sync.dma_start((dbg_eagg if L == 0 else dbg_eagg1)[:, :], eagg[:])
                    if t == 0 and L == 1:
                        hf = sb.tile([P, Kt * 64], f32, name="hf", tag="hf")
                        nc.vector.tensor_copy(out=hf[:], in_=h_st[:])
                        nc.sync.dma_start(dbg_h[:, :], hf[:])
                        scf = sb.tile([P, Kt * 68], f32, name="scf", tag="scf")
                        nc.vector.tensor_copy(out=scf[:], in_=scat[:])
                        nc.sync.dma_start(dbg_scat[:, :], scf[:])
                    if t == 0 and L == 1:
                        gf = sb.tile([P, Kt * 68], f32, name="gf", tag="gf")
                        nc.vector.tensor_copy(out=gf[:], in_=g_t[:])
                        nc.sync.dma_start(dbg_g1[:, :], gf[:])
                    deg1 = sb.tile([P, 1], f32, name="deg", tag="deg")
                    nc.vector.tensor_scalar_max(deg1[:], eagg[:, 64:65], 1.0)
                    inv = sb.tile([P, 1], f32, name="inv", tag="inv")
                    nc.vector.reciprocal(out=inv[:], in_=deg1[:])
                    posn = sb.tile([P, 4], bf16, name="posn", tag="posn")
                    posf = sb.tile([P, 4], f32, name="posf", tag="posf")
                    nc.vector.tensor_scalar_mul(posf[:, 0:3], eagg[:, 65:68], inv[:, 0:1])
                    nc.vector.tensor_tensor(out=posn[:, 0:3], in0=posf[:, 0:3],
                                            in1=pos_own[:, t * 4:t * 4 + 3], op=ALU.add)
                    nc.vector.memset(posn[:, 3:4], 0.0)
                    nc.vector.tensor_copy(out=posn_all[:, t * 4:(t + 1) * 4], in_=posn[:])
                    eaggb = sb.tile([P, 64], bf16, name="eaggb", tag="eaggb")
                    nc.vector.tensor_copy(out=eaggb[:], in_=eagg[:, 0:64])
                    pet = ps_phT.tile([64, P], bf16, name="phT", tag="phT", space="PSUM")
                    nc.tensor.transpose(out=pet[:], in_=eaggb[:], identity=ident[:])
                    eaT = sb.tile([64, P], bf16, name="eaT", tag="eaT")
                    nc.scalar.activation(out=eaT[:], in_=pet[:], func=AF.Copy)
                    pn1 = ps_ph.tile([64, P], f32, name="ph", tag="ph", space="PSUM")
                    nc.tensor.matmul(out=pn1[:], lhsT=W['wn1x' + sfx][:],
                                     rhs=xin_T[:, t * P:(t + 1) * P], start=True, stop=False)
                    nc.tensor.matmul(out=pn1[:], lhsT=W['wn1a' + sfx][:], rhs=eaT[:],
                                     start=False, stop=False)
                    nc.tensor.matmul(out=pn1[:], lhsT=W['nb1_' + sfx][:], rhs=ones_row[:],
                                     start=False, stop=True)
                    zst = sb.tile([65, P], bf16, name="zst", tag="zst")
                    nc.scalar.activation(out=zst[0:64, :], in_=pn1[:], func=AF.Silu)
                    nc.vector.memset(zst[64:65, :], 1.0)
                    px1 = ps_ph.tile([P, P], f32, name="ph", tag="ph", space="PSUM")
                    nc.tensor.matmul(out=px1[:], lhsT=W['wn2b' + sfx][:], rhs=zst[:],
                                     start=True, stop=True)
                    nc.scalar.activation(out=xout_T[:, t * P:(t + 1) * P], in_=px1[:],
                                         func=AF.Copy)
                    if L == 0:
                        pp = ps_ph.tile([P, 64], f32, name="ph", tag="ph", space="PSUM")
                        nc.tensor.matmul(out=pp[:], lhsT=xout_T[:, t * P:(t + 1) * P],
                                         rhs=W['wsrc1'][:], start=True, stop=True)
                        tst = sb.tile([P, 68], bf16, name="tst", tag="tst")
                        nc.vector.tensor_copy(out=tst[:, 0:64], in_=pp[:])
                        nc.vector.tensor_scalar_mul(tst[:, 64:67], posn[:, 0:3], -1.0)
                        nc.vector.memset(tst[:, 67:68], 0.0)
                        nc.sync.dma_start(ts1sh[t * P:(t + 1) * P, :], tst[:])
                        if t == 0:
                            tsf = sb.tile([P, 68], f32, name="tsf", tag="tsf")
                            nc.vector.tensor_copy(out=tsf[:], in_=tst[:])
                            nc.sync.dma_start(dbg_tst[:, :], tsf[:])
                            x1f = sb.tile([P, P], f32, name="x1f", tag="x1f")
                            nc.vector.tensor_copy(out=x1f[:], in_=xout_T[:, 0:P])
                            nc.sync.dma_start(dbg_x1[:, :], x1f[:])
                        pp2 = ps_ph.tile([P, 64], f32, name="ph", tag="ph", space="PSUM")
                        nc.tensor.matmul(out=pp2[:], lhsT=xout_T[:, t * P:(t + 1) * P],
                                         rhs=W['wdst1'][:], start=True, stop=True)
                        xds = sb.tile([P, 72], bf16, name="xds", tag="xds")
                        nc.vector.tensor_copy(out=xds[:, 0:64], in_=pp2[:])
                        nc.vector.tensor_copy(out=xds[:, 64:68], in_=posn[:])
                        nc.vector.memset(xds[:, 68:72], 0.0)
                        nc.sync.dma_start(xdp[1][t * P:(t + 1) * P, :], xds[:])
                    else:
                        bpt = sb.tile([P, 64], bf16, name="bpt", tag="bpt")
                        nc.sync.dma_start(bpt[:], bpool_d[:, t * 64:(t + 1) * 64])
                        pxn = ps_ph.tile([P, P], bf16, name="ph", tag="ph", space="PSUM")
                        nc.tensor.transpose(out=pxn[:], in_=xout_T[:, t * P:(t + 1) * P],
                                            identity=ident[:])
                        x2n = sb.tile([P, P], bf16, name="x2n", tag="x2n")
                        nc.scalar.activation(out=x2n[:], in_=pxn[:], func=AF.Copy)
                        nc.tensor.matmul(out=ppool_t[:], lhsT=bpt[:], rhs=x2n[:],
                                         start=(t == 0), stop=(t == T - 1))

            # layer 0
            tc.strict_bb_all_engine_barrier()
            layer(0, xT_a, xT_b)
            # allgather ts1
            tc.strict_bb_all_engine_barrier()
            nc.gpsimd.collective_compute(
                "AllGather", ALU.bypass, replica_groups=[list(range(NC))],
                ins=[ts1sh.ap().opt()], outs=[ts1.ap().opt()])
            tc.strict_bb_all_engine_barrier()
            # update pos_own from posn_all for layer 1
            nc.vector.tensor_copy(out=pos_own[:], in_=posn_all[:])
            # layer 1 (+ pooling accumulation)
            ppool_t = ps_ppool.tile([G_, P], f32, name="ppool", tag="ppool", space="PSUM")
            layer(1, xT_b, xT_a)
            # pooling tail
            gss = sb.tile([G_, P], f32, name="gss", tag="gss")
            nc.vector.tensor_copy(out=gss[:], in_=ppool_t[:])
            nc.sync.dma_start(gs_in[:, :], gss[:])
            tc.strict_bb_all_engine_barrier()
            nc.gpsimd.collective_compute(
                "AllReduce", ALU.add, replica_groups=[list(range(NC))],
                ins=[gs_in.ap().opt()], outs=[gs_out.ap().opt()])
            tc.strict_bb_all_engine_barrier()
            gsr = sb.tile([G_, P], f32, name="gsr", tag="gsr")
            nc.sync.dma_start(gsr[:], gs_out[:, :])
            gm = sb.tile([G_, P], f32, name="gm", tag="gm")
            nc.vector.tensor_scalar_mul(gm[:], gsr[:], invcnt_t[:, 0:1])
            gr = sb.tile([G_, P], bf16, name="gr", tag="gr")
            nc.scalar.activation(out=gr[:], in_=gm[:], func=AF.Relu)
            pgt = ps_ph.tile([P, G_], bf16, name="ph", tag="ph", space="PSUM")
            nc.tensor.transpose(out=pgt[:, 0:G_], in_=gr[:], identity=ident[0:G_, 0:G_])
            gT = sb.tile([P, G_], bf16, name="gT", tag="gT")
            nc.scalar.activation(out=gT[:], in_=pgt[:, 0:G_], func=AF.Copy)
            po1 = ps_ph.tile([P, G_], f32, name="ph", tag="ph", space="PSUM")
            nc.tensor.matmul(out=po1[:], lhsT=W['wo1'][:], rhs=gT[:], start=True, stop=False)
            nc.tensor.matmul(out=po1[:], lhsT=W['wo1b'][:], rhs=ones_row[:, 0:G_],
                             start=False, stop=True)
            r1 = sb.tile([P, G_], bf16, name="r1", tag="r1")
            nc.scalar.activation(out=r1[:], in_=po1[:], func=AF.Relu)
            po2 = ps_ph.tile([32, G_], f32, name="ph", tag="ph", space="PSUM")
            nc.tensor.matmul(out=po2[:], lhsT=W['wo2'][:], rhs=r1[:], start=True, stop=False)
            nc.tensor.matmul(out=po2[:], lhsT=W['wo2b'][:], rhs=ones_row[:, 0:G_],
                             start=False, stop=True)
            o2 = sb.tile([32, G_], bf16, name="o2", tag="o2")
            nc.scalar.activation(out=o2[:], in_=po2[:], func=AF.Copy)
            pot = ps_ph.tile([G_, 32], bf16, name="ph", tag="ph", space="PSUM")
            nc.tensor.transpose(out=pot[0:G_, 0:32], in_=o2[:], identity=ident[0:32, 0:32])
            oT = sb.tile([G_, 32], f32, name="oT", tag="oT")
            nc.scalar.activation(out=oT[:], in_=pot[0:G_, 0:32], func=AF.Copy)
            nc.sync.dma_start(out_ext[:, :], oT[:])

    return nc


def run(inputs, n_tiles_per_core, trace=False):
    st = host_prep(inputs, n_tiles_per_core)
    w = host_weights(inputs)
    SH = st['SH']
    nc = build(st)
    in_maps = []
    for c in range(NC):
        m = dict(x_own=np.ascontiguousarray(st['xbf'][c * SH:(c + 1) * SH]),
                 pos_own_in=np.ascontiguousarray(st['pbf'][c * SH:(c + 1) * SH]),
                 src_pm=st['src_pm'][c], ea_t=st['ea_t'][c],
                 s_full=st['s_full'][c], st_full=st['st_full'][c],
                 xsT=st['xsT'][c], nsp=st['nsp'][c],
                 bpool=st['bpool'][c], invcnt=st['invcnt'])
        m.update(w)
        in_maps.append(m)
    res = bass_utils.run_bass_kernel_spmd(nc, in_maps, core_ids=list(range(NC)),
                                          trace=trace)
    return res


def kernel(**inputs):
    n_tiles = math.ceil(inputs['x'].shape[0] / (P * NC))
    res = run(inputs, n_tiles)
    return res.results[0]['out']
